# revision 17
# baseline (speedup 1.0000x reference)
"""CROMA (dual-ViT + cross transformer) Trainium2 kernel.

Data-parallel over batch: 16 images -> 8 NeuronCores x 2 images each.
Per core: feature-major activations [D partitions, 450 tokens] (2 x 225),
bf16 matmuls with fp32 PSUM accumulation, partition-axis softmax without
max-subtraction (scores are bounded; alibi bias folded in as a precomputed
exp(bias) multiplier), LayerNorm stats via ones-matmul reductions.

Self-contained: host-side packing (numpy), bass/Tile program builder, SPMD
runner. No files outside this module are read.
"""

import contextlib
import os

import numpy as np
import ml_dtypes

bf16 = ml_dtypes.bfloat16

H = 16          # heads
HD = 48         # head dim
D = 768         # model dim
P = 128         # partitions
NK = D // P     # 6 k-tiles over D
TB = 225        # tokens per image
T = 2 * TB      # tokens per core (2 images)
EPS = 1e-5
N_S1, N_S2, N_CROSS = 6, 12, 6
SCALE = HD ** -0.5

_f32 = np.float32


# ---------------------------------------------------------------------------
# host-side packing
# ---------------------------------------------------------------------------

class Blob:
    def __init__(self, dtype):
        self.dtype = dtype
        self.parts = []
        self.off = {}
        self.n = 0

    def add(self, name, arr):
        arr = np.ascontiguousarray(arr, dtype=self.dtype)
        self.off[name] = self.n
        self.parts.append(arr.reshape(-1))
        self.n += arr.size

    def concat(self):
        return np.concatenate(self.parts) if self.parts else np.zeros(1, self.dtype)


def _blocked(w):
    """[din, dout] -> [nm, 128, nk*128]; lhsT block for (m, k) is
    out[m][:, 128k:128k+128]."""
    din, dout = w.shape
    nk, nm = din // P, dout // P
    return w.reshape(nk, P, nm, P).transpose(2, 1, 0, 3).reshape(nm, P, nk * P)


def _pad_heads(w):
    """[768, 768] -> [768, 1024]: head h cols 48h..48h+48 -> 64h..64h+48."""
    out = np.zeros((D, H * 64), _f32)
    for h in range(H):
        out[:, 64 * h:64 * h + HD] = w[:, HD * h:HD * h + HD]
    return out


def _pad_head_rows(w):
    """[768, 768] -> [1024, 768]: head h rows 48h.. -> 64h.. (zeros in pads)."""
    out = np.zeros((H * 64, D), _f32)
    for h in range(H):
        out[64 * h:64 * h + HD, :] = w[HD * h:HD * h + HD, :]
    return out


def _pk(v, nk):
    """per-feature vector [nk*128] -> [128, nk] partition-major tile."""
    return np.asarray(v, _f32).reshape(nk, P).T


def pack_inputs(SAR_images, optical_images, attn_bias, params):
    wb = Blob(bf16)
    cb = Blob(_f32)

    def add_attn(tag, p, cross=False):
        if cross:
            wq, wk, wv, wo = p['w_q'], p['w_k'], p['w_v'], p['w_out']
        else:
            wq, wk, wv = np.split(np.asarray(p['w_qkv'], _f32), 3, axis=1)
            wo = p['w_out']
        wb.add(tag + ".wq", _blocked(_pad_heads(np.asarray(wq, _f32) * SCALE)))
        wb.add(tag + ".wk", _blocked(_pad_heads(np.asarray(wk, _f32))))
        wb.add(tag + ".wv", np.asarray(wv, _f32))               # natural [768,768]
        wb.add(tag + ".wo", _blocked(_pad_head_rows(np.asarray(wo, _f32))))
        cb.add(tag + ".bo", _pk(p['b_out'], NK))
        cb.add(tag + ".lng", _pk(p['ln_g'], NK))
        cb.add(tag + ".lnb", _pk(p['ln_b'], NK))

    def add_ffn(tag, p):
        wb.add(tag + ".w1", _blocked(np.asarray(p['w1'], _f32)))
        wb.add(tag + ".w2", _blocked(np.asarray(p['w2'], _f32)))
        cb.add(tag + ".b1", _pk(p['b1'], 4 * NK))
        cb.add(tag + ".b2", _pk(p['b2'], NK))
        cb.add(tag + ".lng", _pk(p['ln_g'], NK))
        cb.add(tag + ".lnb", _pk(p['ln_b'], NK))

    s1, s2, cr = params['s1'], params['s2'], params['cross']
    wb.add("s1.win", _blocked(np.asarray(s1['w_in'], _f32)))    # [128 -> 768]
    cb.add("s1.bin", _pk(s1['b_in'], NK))
    wb.add("s2.win", _blocked(np.asarray(s2['w_in'], _f32)))
    cb.add("s2.bin", _pk(s2['b_in'], NK))
    for i, lp in enumerate(s1['layers']):
        add_attn(f"s1.{i}.a", lp['attn']); add_ffn(f"s1.{i}.f", lp['ffn'])
    cb.add("s1.n.lng", _pk(s1['norm']['g'], NK))
    cb.add("s1.n.lnb", _pk(s1['norm']['b'], NK))
    for i, lp in enumerate(s2['layers']):
        add_attn(f"s2.{i}.a", lp['attn']); add_ffn(f"s2.{i}.f", lp['ffn'])
    cb.add("s2.n.lng", _pk(s2['norm']['g'], NK))
    cb.add("s2.n.lnb", _pk(s2['norm']['b'], NK))
    for i, lp in enumerate(cr['layers']):
        add_attn(f"cr.{i}.a", lp['attn'])
        add_attn(f"cr.{i}.c", lp['cross'], cross=True)
        add_ffn(f"cr.{i}.f", lp['ffn'])
    cb.add("cr.n.lng", _pk(cr['norm']['g'], NK))
    cb.add("cr.n.lnb", _pk(cr['norm']['b'], NK))

    # exp(alibi bias), transposed j-tile layout [2][128][16*225]
    eb = np.asarray(attn_bias, _f32)[0]                          # [H, 225, 225]
    ebias = np.zeros((2, P, H * TB), _f32)
    for jt, (j0, jsz) in enumerate(((0, 128), (128, 97))):
        for h in range(H):
            ebias[jt, :jsz, h * TB:(h + 1) * TB] = np.exp(eb[h, j0:j0 + jsz, :])
    shared = {
        "wb": wb.concat(),
        "cb": cb.concat(),
        "ebias": ebias.astype(bf16),
    }

    def patchify(imgs):
        b, c, hh, ww = imgs.shape
        x = imgs.reshape(b, c, hh // 8, 8, ww // 8, 8).transpose(0, 2, 4, 1, 3, 5)
        return x.reshape(b, (hh // 8) * (ww // 8), c * 64)

    sar_p = patchify(np.asarray(SAR_images, _f32))               # [16, 225, 128]
    opt_p = patchify(np.asarray(optical_images, _f32))           # [16, 225, 768]
    per_core = []
    for core in range(8):
        sl = slice(2 * core, 2 * core + 2)
        px_sar = sar_p[sl].reshape(T, 2 * 64).T                  # [128, 450]
        px_opt = opt_p[sl].reshape(T, 12 * 64).T                 # [768, 450]
        per_core.append({
            "px_sar": np.ascontiguousarray(px_sar).astype(bf16),
            "px_opt": np.ascontiguousarray(px_opt).astype(bf16),
            **shared,
        })
    return wb, cb, per_core


# ---------------------------------------------------------------------------
# program builder
# ---------------------------------------------------------------------------

def _row_segs(r0, r1):
    """global feature rows [r0, r1) -> [(tile, base, size, src_off)]"""
    segs, r = [], r0
    while r < r1:
        t, b = r // P, r % P
        sz = min(r1 - r, P - b)
        segs.append((t, b, sz, r - r0))
        r += sz
    return segs


def _jtk(b, jt):
    """(token col start, size) for batch b, j-tile jt"""
    return (TB * b + 128 * jt, 128 if jt == 0 else TB - 128)


def build_program(wb, cb, n_s1=N_S1, n_s2=N_S2, n_cross=N_CROSS):
    ABL = set(os.environ.get("KABL", "").split(","))
    import concourse.bass as bass
    import concourse.mybir as mybir
    import concourse.tile as tile

    dt = mybir.dt
    AF = mybir.ActivationFunctionType
    OP = mybir.AluOpType

    nc = bass.Bass(trn_type="TRN2", target_bir_lowering=False, debug=False)
    px_sar = nc.declare_dram_parameter("px_sar", [P, T], dt.bfloat16, isOutput=False)
    px_opt = nc.declare_dram_parameter("px_opt", [D, T], dt.bfloat16, isOutput=False)
    wb_d = nc.declare_dram_parameter("wb", [wb.n], dt.bfloat16, isOutput=False)
    cb_d = nc.declare_dram_parameter("cb", [cb.n], dt.float32, isOutput=False)
    eb_d = nc.declare_dram_parameter("ebias", [2, P, H * TB], dt.bfloat16, isOutput=False)
    out_d = nc.declare_dram_parameter("out", [D, T], dt.float32, isOutput=True)

    def wslice(name, m, nk):
        """blocked lhsT layout: m-tile view [128, nk*128]"""
        off = wb.off[name] + m * P * nk * P
        return wb_d.ap()[off:off + P * nk * P].rearrange("(p c) -> p c", c=nk * P)

    def wnat(name, k):
        """natural [768, dout] weight: rows [128k, 128k+128) as [128, dout]"""
        off = wb.off[name] + k * P * D
        return wb_d.ap()[off:off + P * D].rearrange("(p c) -> p c", c=D)

    def cslice(name, nk):
        off = cb.off[name]
        return cb_d.ap()[off:off + P * nk].rearrange("(p c) -> p c", c=nk)

    ctx = contextlib.ExitStack()
    with tile.TileContext(nc) as tc:
        persist = ctx.enter_context(tc.tile_pool(name="persist", bufs=1))
        wpool6 = ctx.enter_context(tc.tile_pool(name="wpool6", bufs=3))
        wpool24 = ctx.enter_context(tc.tile_pool(name="wpool24", bufs=2))
        mbrb = ctx.enter_context(tc.tile_pool(name="mbrb", bufs=2))
        wvpool = ctx.enter_context(tc.tile_pool(name="wvpool", bufs=6))
        lnpool = ctx.enter_context(tc.tile_pool(name="lnpool", bufs=3))
        tmp = ctx.enter_context(tc.tile_pool(name="tmp", bufs=4))
        ring = ctx.enter_context(tc.tile_pool(name="ring", bufs=30))
        stat = ctx.enter_context(tc.tile_pool(name="stat", bufs=1))
        ps = ctx.enter_context(tc.tile_pool(name="ps", bufs=8, space="PSUM"))

        _ring_n = [0]

        def rtile():
            _ring_n[0] += 1
            return ring.tile([P, T], dt.bfloat16, tag="bbf", name=f"bbf{_ring_n[0]}")

        def pt(tag, shape, dtype):
            return persist.tile(shape, dtype, tag=tag, name=tag)

        xT = [pt(f"xT{k}", [P, T], dt.float32) for k in range(NK)]
        sarT = [pt(f"sarT{k}", [P, T], dt.float32) for k in range(NK)]
        ctxT = [pt(f"ctxT{k}", [P, T], dt.float32) for k in range(NK)]
        vT = [pt(f"vT{k}", [P, D], dt.bfloat16) for k in range(4)]
        oT = [pt(f"oT{k}", [P, T], dt.bfloat16) for k in range(8)]
        for t in oT:  # pad rows (48:64, 112:128) are read by out-proj matmuls
            nc.vector.memset(t[:], 0.0)
        a_un = [[pt(f"au{b}{j}", [P, H * TB], dt.bfloat16) for j in range(2)]
                for b in range(2)]
        ebias = [pt(f"eb{j}", [P, H * TB], dt.bfloat16) for j in range(2)]
        rdbc = [pt(f"rdbc{b}", [P, H * TB], dt.bfloat16) for b in range(2)]
        ones_bf = pt("ones_bf", [P, 1], dt.bfloat16)
        onesrow = pt("onesrow", [1, P], dt.float32)
        onesrow_bf = pt("onesrow_bf", [1, P], dt.bfloat16)
        eps_t = pt("eps", [1, 1], dt.float32)
        nc.vector.memset(ones_bf[:], 1.0)
        nc.vector.memset(onesrow[:], 1.0)
        nc.vector.memset(onesrow_bf[:], 1.0)
        nc.vector.memset(eps_t[:], EPS)
        for j in range(2):
            nc.sync.dma_start(out=ebias[j][:], in_=eb_d.ap()[j])

        # ---------------- helpers ----------------

        def psum(n=T):
            return ps.tile([P, 512], dt.float32, tag="ps", name="psb")[:, :n]

        def ln(src, tag, out_tiles):
            """LayerNorm over features (partition axis); out dtype = tile dtype."""
            lng = lnpool.tile([P, NK], dt.float32, tag="lng")
            nc.sync.dma_start(out=lng[:], in_=cslice(tag + ".lng", NK))
            lnb = lnpool.tile([P, NK], dt.float32, tag="lnb")
            nc.sync.dma_start(out=lnb[:], in_=cslice(tag + ".lnb", NK))
            ps_s = psum()[0:1, :]
            ps_q = psum()[0:1, :]
            xbfs, xsqs = [], []
            for k in range(NK):
                xbf = tmp.tile([P, T], dt.bfloat16, tag="xbf")
                nc.vector.tensor_copy(out=xbf[:], in_=src[k][:])
                xbfs.append(xbf)
                xsq = tmp.tile([P, T], dt.bfloat16, tag="xsq")
                nc.vector.tensor_mul(out=xsq[:], in0=xbf[:], in1=xbf[:])
                xsqs.append(xsq)
            for k in range(NK):
                nc.tensor.matmul(ps_s, lhsT=ones_bf[:], rhs=xbfs[k][:],
                                 start=(k == 0), stop=(k == NK - 1))
            for k in range(NK):
                nc.tensor.matmul(ps_q, lhsT=ones_bf[:], rhs=xsqs[k][:],
                                 start=(k == 0), stop=(k == NK - 1))
            mean = stat.tile([1, T], dt.float32, tag="mean")
            nc.vector.tensor_scalar_mul(out=mean[:], in0=ps_s, scalar1=1.0 / D)
            msq = stat.tile([1, T], dt.float32, tag="msq")
            nc.vector.tensor_scalar_mul(out=msq[:], in0=ps_q, scalar1=1.0 / D)
            var = stat.tile([1, T], dt.float32, tag="var")
            nc.vector.tensor_mul(out=var[:], in0=mean[:], in1=mean[:])
            nc.vector.tensor_tensor(out=var[:], in0=msq[:], in1=var[:],
                                    op=OP.subtract)
            std = stat.tile([1, T], dt.float32, tag="std")
            nc.scalar.activation(out=std[:], in_=var[:], func=AF.Sqrt, bias=eps_t[:])
            rstd = stat.tile([1, T], dt.float32, tag="rstd")
            nc.vector.reciprocal(out=rstd[:], in_=std[:])
            ps_mb = psum()
            nc.tensor.matmul(ps_mb, lhsT=onesrow[:], rhs=mean[:], start=True,
                             stop=True)
            ps_rb = psum()
            nc.tensor.matmul(ps_rb, lhsT=onesrow[:], rhs=rstd[:], start=True,
                             stop=True)
            mb = mbrb.tile([P, T], dt.float32, tag="mb")
            nc.scalar.activation(out=mb[:], in_=ps_mb, func=AF.Copy)
            rb = mbrb.tile([P, T], dt.float32, tag="rb")
            nc.scalar.activation(out=rb[:], in_=ps_rb, func=AF.Copy)
            for k in range(NK):
                t1 = tmp.tile([P, T], dt.float32, tag="sc32", name="t1")
                nc.vector.tensor_tensor(out=t1[:], in0=src[k][:], in1=mb[:],
                                        op=OP.subtract)
                nc.vector.tensor_mul(out=t1[:], in0=t1[:], in1=rb[:])
                nc.scalar.activation(out=out_tiles[k][:], in_=t1[:], func=AF.Identity,
                                     bias=lnb[:, k:k + 1], scale=lng[:, k:k + 1])

        def linear_fm(wname, nm, nk, rhs_tiles, consumer):
            """out[m] = sum_k w[k,m].T @ rhs[k]; consumer(m, psum [128, T])."""
            for m in range(nm):
                wp = wpool24 if nk > NK else wpool6
                wsb = wp.tile([P, nk * P], dt.bfloat16, tag=f"w{nk}", name=f"w{nk}")
                nc.sync.dma_start(out=wsb[:], in_=wslice(wname, m, nk))
                psm = psum()
                for k in range(nk):
                    nc.tensor.matmul(psm, lhsT=wsb[:, P * k:P * (k + 1)],
                                     rhs=rhs_tiles[k][:],
                                     start=(k == 0), stop=(k == nk - 1))
                consumer(m, psm)

        def copy_consumer(dst):
            def c(m, psm):
                nc.scalar.activation(out=dst[m][:], in_=psm, func=AF.Copy)
            return c

        def residual_consumer(dst, btile):
            def c(m, psm):
                t1 = tmp.tile([P, T], dt.float32, tag="sc32", name="res")
                nc.scalar.activation(out=t1[:], in_=psm, func=AF.Identity,
                                     bias=btile[:, m:m + 1])
                nc.vector.tensor_add(out=dst[m][:], in0=dst[m][:], in1=t1[:])
            return c

        def attention(tag, x_src, ctx_src):
            if "noattn" in ABL:
                return
            qT = [rtile() for _ in range(8)]
            kT = [rtile() for _ in range(8)]
            xn = [rtile() for _ in range(NK)]
            ln(x_src, tag, xn)
            linear_fm(tag + ".wq", 8, NK, xn, copy_consumer(qT))
            if ctx_src is not x_src:
                xn = [rtile() for _ in range(NK)]
                ln(ctx_src, tag, xn)
            linear_fm(tag + ".wk", 8, NK, xn, copy_consumer(kT))
            # v token-major: lhsT = activations, rhs = natural weight k-slices
            wv = []
            for k in range(NK):
                w = wvpool.tile([P, D], dt.bfloat16, tag="wv")
                nc.sync.dma_start(out=w[:], in_=wnat(tag + ".wv", k))
                wv.append(w)
            for t in range(4):
                b, jt = t // 2, t % 2
                c0, csz = _jtk(b, jt)
                psa = psum(384)[:csz, :]
                psb = psum(384)[:csz, :]
                for k in range(NK):
                    nc.tensor.matmul(psa, lhsT=xn[k][:, c0:c0 + csz],
                                     rhs=wv[k][:, 0:384],
                                     start=(k == 0), stop=(k == NK - 1))
                for k in range(NK):
                    nc.tensor.matmul(psb, lhsT=xn[k][:, c0:c0 + csz],
                                     rhs=wv[k][:, 384:768],
                                     start=(k == 0), stop=(k == NK - 1))
                nc.scalar.activation(out=vT[t][0:csz, 0:384], in_=psa, func=AF.Copy)
                nc.scalar.activation(out=vT[t][0:csz, 384:768], in_=psb,
                                     func=AF.Copy)
            # scores + exp + ebias (head pairs share one psum bank)
            for b in range(2):
                if "noscores" in ABL:
                    break
                qc0 = TB * b
                for h in range(H):
                    for jt in range(2):
                        j0, jsz = _jtk(b, jt)
                        psp = ps.tile([P, 512], dt.float32, tag="ps", name="psb")
                        ht, hb = h // 2, 64 * (h % 2)
                        nc.tensor.matmul(
                            psp[0:jsz, 0:TB],
                            lhsT=kT[ht][hb:hb + HD, j0:j0 + jsz],
                            rhs=qT[ht][hb:hb + HD, qc0:qc0 + TB],
                            start=True, stop=True)
                        asl = a_un[b][jt][0:jsz, TB * h:TB * (h + 1)]
                        nc.scalar.activation(out=asl, in_=psp[0:jsz, 0:TB],
                                             func=AF.Exp)
                        nc.vector.tensor_mul(
                            out=asl, in0=asl,
                            in1=ebias[jt][0:jsz, TB * h:TB * (h + 1)])
                # den per head -> reciprocal
                for h in range(H):
                    if "noden" in ABL:
                        break
                    psd = psum(TB)[0:1, :]
                    for jt in range(2):
                        j0, jsz = _jtk(b, jt)
                        nc.tensor.matmul(
                            psd, lhsT=ones_bf[0:jsz, :],
                            rhs=a_un[b][jt][0:jsz, TB * h:TB * (h + 1)],
                            start=(jt == 0), stop=(jt == 1))
                    with nc.allow_low_precision(reason="rden bf16 feeds bf16 bcast matmul"):
                        nc.vector.reciprocal(out=rdbc[b][0:1, TB * h:TB * (h + 1)],
                                             in_=psd)
                # broadcast rden across partitions (K=1 matmuls); normalize a
                for c in range(8):
                    if "noden" in ABL or "nobc" in ABL:
                        break
                    n0 = 450 * c
                    psr = psum(450)
                    nc.tensor.matmul(psr, lhsT=onesrow_bf[:],
                                     rhs=rdbc[b][0:1, n0:n0 + 450],
                                     start=True, stop=True)
                    nc.scalar.activation(out=rdbc[b][:, n0:n0 + 450], in_=psr,
                                         func=AF.Copy)
                for jt in range(2):
                    if "noden" in ABL or "nobc" in ABL or "nonorm" in ABL:
                        break
                    j0, jsz = _jtk(b, jt)
                    nc.vector.tensor_mul(out=a_un[b][jt][0:jsz, :],
                                         in0=a_un[b][jt][0:jsz, :],
                                         in1=rdbc[b][0:jsz, :])
            # a @ v -> oT (one psum bank per (h, b): single accumulation group)
            for h in range(H):
                if "noav" in ABL or "noscores" in ABL:
                    break
                hb = 64 * (h % 2)
                for b in range(2):
                    pso = ps.tile([P, 512], dt.float32, tag="ps", name="psb")
                    for jt in range(2):
                        j0, jsz = _jtk(b, jt)
                        nc.tensor.matmul(
                            pso[0:HD, 0:TB],
                            lhsT=vT[2 * b + jt][0:jsz, HD * h:HD * (h + 1)],
                            rhs=a_un[b][jt][0:jsz, TB * h:TB * (h + 1)],
                            start=(jt == 0), stop=(jt == 1))
                    nc.scalar.activation(
                        out=oT[h // 2][hb:hb + HD, TB * b:TB * (b + 1)],
                        in_=pso[0:HD, 0:TB], func=AF.Copy)
            bo = lnpool.tile([P, NK], dt.float32, tag="bo")
            nc.sync.dma_start(out=bo[:], in_=cslice(tag + ".bo", NK))
            linear_fm(tag + ".wo", NK, 8, oT, residual_consumer(x_src, bo))

        def ffn(tag, x_src):
            if "noffn" in ABL:
                return
            xn = [rtile() for _ in range(NK)]
            hT = [rtile() for _ in range(4 * NK)]
            ln(x_src, tag, xn)
            b1 = lnpool.tile([P, 4 * NK], dt.float32, tag="b1")
            nc.sync.dma_start(out=b1[:], in_=cslice(tag + ".b1", 4 * NK))

            def gelu_c(m, psm):
                f = AF.Identity if "nogelu" in ABL else AF.Gelu
                nc.scalar.activation(out=hT[m][:], in_=psm, func=f,
                                     bias=b1[:, m:m + 1])
            linear_fm(tag + ".w1", 4 * NK, NK, xn, gelu_c)
            b2 = lnpool.tile([P, NK], dt.float32, tag="b2")
            nc.sync.dma_start(out=b2[:], in_=cslice(tag + ".b2", NK))
            linear_fm(tag + ".w2", NK, 4 * NK, hT, residual_consumer(x_src, b2))

        def patch_embed(wname, px_tiles, nk, bias_name):
            bi = lnpool.tile([P, NK], dt.float32, tag="bin")
            nc.sync.dma_start(out=bi[:], in_=cslice(bias_name, NK))

            def c(m, psm):
                nc.scalar.activation(out=xT[m][:], in_=psm, func=AF.Identity,
                                     bias=bi[:, m:m + 1])
            linear_fm(wname, NK, nk, px_tiles, c)

        # ---------------- model ----------------
        pxs = stat.tile([P, T], dt.bfloat16, tag="pxs")
        nc.sync.dma_start(out=pxs[:], in_=px_sar.ap())
        patch_embed("s1.win", [pxs], 1, "s1.bin")
        for i in range(n_s1):
            attention(f"s1.{i}.a", xT, xT)
            ffn(f"s1.{i}.f", xT)
        ln(xT, "s1.n", sarT)

        pxo = [stat.tile([P, T], dt.bfloat16, tag=f"pxo{k}", name=f"pxo{k}") for k in range(NK)]
        for k in range(NK):
            nc.sync.dma_start(out=pxo[k][:], in_=px_opt.ap()[P * k:P * (k + 1), :])
        patch_embed("s2.win", pxo, NK, "s2.bin")
        for i in range(n_s2):
            attention(f"s2.{i}.a", xT, xT)
            ffn(f"s2.{i}.f", xT)
        ln(xT, "s2.n", ctxT)

        for i in range(n_cross):
            attention(f"cr.{i}.a", sarT, sarT)
            attention(f"cr.{i}.c", sarT, ctxT)
            ffn(f"cr.{i}.f", sarT)
        class _OutSink:
            def __getitem__(self, k):
                t = tmp.tile([P, T], dt.float32, tag="res", name=f"outt{k}")
                nc.sync.dma_start(out=out_d.ap()[P * k:P * (k + 1), :], in_=t[:])
                return _Deferred(t)

        class _Deferred:
            def __init__(self, t):
                self.t = t

            def __getitem__(self, sl):
                return self.t[sl]

        # ln writes out_tiles[k][:] then we DMA; emit DMA after the write by
        # allocating tiles up front and DMAing post-ln instead:
        outt = [tmp.tile([P, T], dt.float32, tag="sc32", name=f"outt{k}") for k in range(NK)]
        ln(sarT, "cr.n", outt)
        for k in range(NK):
            nc.sync.dma_start(out=out_d.ap()[P * k:P * (k + 1), :], in_=outt[k][:])

        ctx.close()
    return nc


# ---------------------------------------------------------------------------
# runtime glue
# ---------------------------------------------------------------------------

def _install_drain_patch():
    """walrus rejects instructions carrying more than a few sem waits; split
    the Tile kernel-tail drain into one drain per waited proc, and excess
    per-instruction waits onto preceding EventSemaphore carriers."""
    import bass_rust
    import concourse.mybir as mybir
    from concourse import tile as tile_mod
    from concourse.vector_clock import ScopedClock, VectorClock
    if getattr(tile_mod.TileContext, "_drain_patched", False):
        return

    MAXW = 1          # walrus limit per compute instruction
    EVSW = 2          # waits per EventSemaphore carrier
    _orig_postorder = tile_mod.postorder_instruction_blocks
    _ctr = [0]

    def _split_waits(obb):
        for bname, insts in obb.items():
            out = []
            for inst in insts:
                si = inst.sync_info
                waits = list(si.on_wait) if si and si.on_wait else []
                if len(waits) > MAXW:
                    keep = waits[:MAXW]
                    extra = waits[MAXW:]
                    while extra:
                        chunk, extra = extra[:EVSW], extra[EVSW:]
                        _ctr[0] += 1
                        evs = mybir.InstEventSemaphore(name=f"evs-split-{_ctr[0]}")
                        evs.engine = inst.engine
                        evs.sync_info = bass_rust.SyncInfo(on_wait=chunk,
                                                           on_update=[])
                        for attr in ("bass_scheduled_tick", "bass_scheduled_proc",
                                     "bass_scheduled_scope"):
                            try:
                                setattr(evs, attr, getattr(inst, attr))
                            except Exception:
                                pass
                        out.append(evs)
                    inst.sync_info = bass_rust.SyncInfo(
                        on_wait=keep, on_update=list(si.on_update or []))
                out.append(inst)
            obb[bname] = out

    def _patched_postorder(obb, start_bb_name, postordered):
        _split_waits(obb)
        return _orig_postorder(obb, start_bb_name, postordered)

    tile_mod.postorder_instruction_blocks = _patched_postorder

    def _patched(self, tick_clock, wait_clock):
        nc = self.nc
        full = list(tick_clock.global_clock)
        for i in [i for i, t in enumerate(full) if t > 0]:
            sub = [full[j] if j == i else 0 for j in range(len(full))]
            d = nc.sync.drain()
            wait_clock.add_sem_waits(d.ins, ScopedClock({None: VectorClock(sub)}))
        nc.all_engine_barrier()
        assert self.sems is not None
        popped = nc._tile_sem_poison_stack.pop()
        assert popped is self._sem_poison
        nc.clear_and_free_semaphores(list(self.sems.allocated().values()))
        nc.all_engine_barrier()

    tile_mod.TileContext._drain_and_barrier = _patched
    tile_mod.TileContext._drain_patched = True


_PROG_CACHE = {}


def get_program(wb, cb, n_s1=N_S1, n_s2=N_S2, n_cross=N_CROSS):
    key = (n_s1, n_s2, n_cross)
    if key not in _PROG_CACHE:
        _install_drain_patch()
        _PROG_CACHE[key] = build_program(wb, cb, n_s1, n_s2, n_cross)
    return _PROG_CACHE[key]


def unpack_output(results):
    outs = []
    for r in results:
        yT = r["out"]                       # [768, 450] f32
        outs.append(np.ascontiguousarray(yT.T).reshape(2, TB, D))
    return np.concatenate(outs, axis=0).astype(np.float32)


last_results = None


def kernel(SAR_images, optical_images, attn_bias, params):
    global last_results
    from concourse.bass_utils import run_bass_kernel_spmd
    wb, cb, per_core = pack_inputs(SAR_images, optical_images, attn_bias, params)
    nc = get_program(wb, cb)
    trace = bool(os.environ.get("KTRACE"))
    res = run_bass_kernel_spmd(nc, per_core, core_ids=list(range(8)), trace=trace)
    last_results = res
    return unpack_output(res.results)
